# revision 1
# baseline (speedup 1.0000x reference)
"""3x3 neighborhood cosine-similarity sum (minus self) on 8 TRN2 NeuronCores.

Input:  input_image [1024, 1024, 1, 128] float32  (H, W, 1, C)
Output: sim [1024, 1024] float32

Algorithm per pixel: sim = <xn, BoxSum3x3(xn)> - 1, where xn = x / max(||x||, eps).

Sharding: H rows split 128/core across 8 cores; each core gets its 128 rows
plus 1 halo row above/below (zero rows at the image edges), i.e. [130, 1024, 128].

Per-core layout ("layout A"): SBUF tiles [128 part = w%128, free = (j=w//128, c)].
 - ss      : per-chunk fused tensor_tensor_reduce (x*x, add-accum) on DVE
 - inv     : sqrt(ss + 1e-16) on ACT, reciprocal on DVE
 - xn      : per-chunk tensor_scalar mult (f32 in, bf16 out) on DVE
 - vertical: 2 tensor_tensor adds (bf16) on DVE over the xn row ring
 - horizontal: band-matrix matmuls on PE (tridiag within chunk + 2 cross-chunk
   single-entry matrices), accumulated in PSUM
 - evac    : PSUM -> SBUF bf16 copy on ACT
 - dot     : per-chunk fused tensor_tensor_reduce (xn*S, add-accum, init=-1) on DVE
Output rows staged as [blk, p, j*16+rr] and untangled on the host.
"""

import numpy as np
import ml_dtypes

import sys

for _p in ("/opt/trn_rl_repo",):
    if _p not in sys.path:
        sys.path.insert(0, _p)

import concourse.bass as bass
import concourse.bacc as bacc
import concourse.mybir as mybir
import concourse.tile as tile
from concourse.bass_utils import run_bass_kernel_spmd

F32 = mybir.dt.float32
BF16 = mybir.dt.bfloat16
ALU = mybir.AluOpType
ACTF = mybir.ActivationFunctionType

H, W, C = 1024, 1024, 128
NCORES = 8
ROWS_PER_CORE = H // NCORES          # 128
NJ = W // 128                        # 8 w-chunks
RBLK = 16                            # output rows per staging block


def build_consts():
    """Host-side constant matrices for the horizontal box-sum matmuls."""
    t = np.zeros((128, 128), np.float32)
    for k in range(128):
        for m in (k - 1, k, k + 1):
            if 0 <= m < 128:
                t[k, m] = 1.0
    el = np.zeros((128, 128), np.float32)
    el[127, 0] = 1.0
    er = np.zeros((128, 128), np.float32)
    er[0, 127] = 1.0
    to_bf = lambda a: a.astype(ml_dtypes.bfloat16)
    return to_bf(t), to_bf(el), to_bf(er)


def build_bass(n_out_rows=ROWS_PER_CORE):
    """Build the per-core Bass graph. n_out_rows output rows need
    n_out_rows + 2 input rows (zero-padded halo included by the host)."""
    n_in = n_out_rows + 2
    nblk = (n_out_rows + RBLK - 1) // RBLK

    nc = bacc.Bacc(None, target_bir_lowering=False)
    x_dram = nc.declare_dram_parameter("x", [n_in, W, C], F32, isOutput=False)
    band_dram = nc.declare_dram_parameter("band", [128, 128], BF16, isOutput=False)
    el_dram = nc.declare_dram_parameter("el", [128, 128], BF16, isOutput=False)
    er_dram = nc.declare_dram_parameter("er", [128, 128], BF16, isOutput=False)
    out_dram = nc.declare_dram_parameter(
        "out", [nblk, 128, NJ * RBLK], F32, isOutput=True
    )

    with tile.TileContext(nc) as tc:
        with (
            tc.tile_pool(name="consts", bufs=1) as cpool,
            tc.tile_pool(name="xin", bufs=3) as xpool,
            tc.tile_pool(name="sq", bufs=2) as sqpool,
            tc.tile_pool(name="norm", bufs=3) as npool,
            tc.tile_pool(name="xn", bufs=5) as xnpool,
            tc.tile_pool(name="v", bufs=2) as vpool,
            tc.tile_pool(name="sb", bufs=2) as sbpool,
            tc.tile_pool(name="pd", bufs=2) as pdpool,
            tc.tile_pool(name="sim", bufs=2) as simpool,
            tc.tile_pool(name="psum", bufs=2, space="PSUM") as psumpool,
        ):
            band = cpool.tile([128, 128], BF16, tag="band")
            el = cpool.tile([128, 128], BF16, tag="el")
            er = cpool.tile([128, 128], BF16, tag="er")
            nc.sync.dma_start(band[:], band_dram[:])
            nc.sync.dma_start(el[:], el_dram[:])
            nc.sync.dma_start(er[:], er_dram[:])
            eps_bias = cpool.tile([128, 1], F32, tag="eps")
            nc.gpsimd.memset(eps_bias[:], 1e-16)

            xn_rows = [None] * n_in
            simt = None

            for h in range(n_in):
                # ---- load row h: [128 p, NJ j, 128 c], w = j*128 + p
                xt = xpool.tile([128, NJ, C], F32, tag="xt")
                nc.sync.dma_start(
                    xt[:], x_dram[h].rearrange("(j p) c -> p j c", p=128)
                )

                # ---- ss[p, j] = sum_c x^2 (fused mult+reduce per chunk)
                sq = sqpool.tile([128, NJ, C], BF16, tag="sq")
                ssr = npool.tile([128, NJ], F32, tag="ssr")
                import os
                if os.environ.get("SS_POW", "0") == "1":
                    for j in range(NJ):
                        nc.vector.tensor_scalar(
                            sq[:, j, :],
                            xt[:, j, :],
                            2.0,
                            0.0,
                            ALU.pow,
                            ALU.add,
                            accum_out=ssr[:, j : j + 1],
                        )
                else:
                    for j in range(NJ):
                        nc.vector.scalar_tensor_tensor(
                            sq[:, j, :],
                            xt[:, j, :],
                            1.0,
                            xt[:, j, :],
                            ALU.mult,
                            ALU.mult,
                            accum_out=ssr[:, j : j + 1],
                        )

                # ---- inv = 1 / sqrt(ss + 1e-16)   (1e-16 keeps zero rows finite,
                #      matches reference x / max(||x||, 1e-8) exactly for zeros)
                snorm = npool.tile([128, NJ], F32, tag="snorm")
                nc.scalar.activation(snorm[:], ssr[:], ACTF.Sqrt, bias=eps_bias[:])
                sinv = npool.tile([128, NJ], F32, tag="sinv")
                nc.vector.reciprocal(sinv[:], snorm[:])

                # ---- xn = x * inv  (f32 -> bf16), per chunk (per-partition scalar)
                xnt = xnpool.tile([128, NJ, C], BF16, tag="xnt")
                for j in range(NJ):
                    nc.vector.tensor_scalar(
                        xnt[:, j, :],
                        xt[:, j, :],
                        sinv[:, j : j + 1],
                        None,
                        ALU.mult,
                    )
                xn_rows[h] = xnt

                if h < 2:
                    continue

                # ---- output row r (padded coords); local output index r-1
                r = h - 1
                ro = r - 1  # 0..n_out_rows-1
                xa, xb_, xc = xn_rows[r - 1], xn_rows[r], xn_rows[r + 1]
                xn_rows[r - 1] = None

                vtmp = vpool.tile([128, NJ, C], BF16, tag="vtmp")
                nc.vector.tensor_add(vtmp[:], xa[:], xc[:])
                vt = vpool.tile([128, NJ, C], BF16, tag="vt")
                nc.vector.tensor_add(vt[:], vtmp[:], xb_[:])

                # ---- horizontal box sum on PE: S = T@V + EL@V(j-1) + ER@V(j+1)
                S = psumpool.tile([128, NJ, C], F32, tag="S")
                hj = NJ // 2  # PSUM bank boundary at j=4 (512 f32)
                nc.tensor.matmul(
                    S[:, 0:hj, :], band[:], vt[:, 0:hj, :], start=True, stop=False
                )
                nc.tensor.matmul(
                    S[:, hj:NJ, :], band[:], vt[:, hj:NJ, :], start=True, stop=False
                )
                nc.tensor.matmul(
                    S[:, 1:hj, :], el[:], vt[:, 0 : hj - 1, :], start=False, stop=False
                )
                nc.tensor.matmul(
                    S[:, hj:NJ, :], el[:], vt[:, hj - 1 : NJ - 1, :],
                    start=False, stop=False,
                )
                nc.tensor.matmul(
                    S[:, 0:hj, :], er[:], vt[:, 1 : hj + 1, :], start=False, stop=True
                )
                nc.tensor.matmul(
                    S[:, hj : NJ - 1, :], er[:], vt[:, hj + 1 : NJ, :],
                    start=False, stop=True,
                )

                # ---- evacuate S to SBUF as bf16 (ACT)
                sb = sbpool.tile([128, NJ, C], BF16, tag="sbt")
                nc.scalar.activation(sb[:], S[:], ACTF.Copy)

                # ---- sim[p, j] = sum_c xn*S - 1 (fused, init = -1)
                if ro % RBLK == 0:
                    simt = simpool.tile([128, NJ * RBLK], F32, tag="simt")
                rr = ro % RBLK
                pd = pdpool.tile([128, NJ, C], BF16, tag="pd")
                for j in range(NJ):
                    col = j * RBLK + rr
                    nc.vector.scalar_tensor_tensor(
                        pd[:, j, :],
                        xb_[:, j, :],
                        1.0,
                        sb[:, j, :],
                        ALU.mult,
                        ALU.mult,
                        accum_out=simt[:, col : col + 1],
                    )

                if ro % RBLK == RBLK - 1 or ro == n_out_rows - 1:
                    blk = ro // RBLK
                    simo = simpool.tile([128, NJ * RBLK], F32, tag="simo")
                    nc.vector.tensor_scalar(
                        simo[:], simt[:], -1.0, None, ALU.add
                    )
                    nc.sync.dma_start(out_dram[blk], simo[:])

    nc.compile()
    return nc


def shard_inputs(input_image):
    """input_image [H, W, 1, C] f32 -> per-core in_maps."""
    x = np.asarray(input_image).reshape(H, W, C).astype(np.float32, copy=False)
    xp = np.zeros((H + 2, W, C), np.float32)
    xp[1 : H + 1] = x
    band, el, er = build_consts()
    in_maps = []
    for core in range(NCORES):
        lo = core * ROWS_PER_CORE
        shard = np.ascontiguousarray(xp[lo : lo + ROWS_PER_CORE + 2])
        in_maps.append({"x": shard, "band": band, "el": el, "er": er})
    return in_maps


def unshard_output(results):
    """results[i]['out'] [nblk, 128, NJ*RBLK] -> [H, W] f32."""
    out = np.empty((H, W), np.float32)
    for core in range(NCORES):
        st = np.asarray(results[core]["out"])  # [nblk, 128, NJ*RBLK]
        nblk = st.shape[0]
        st = st.reshape(nblk, 128, NJ, RBLK)  # [blk, p, j, rr]
        sim = st.transpose(0, 3, 2, 1).reshape(nblk * RBLK, W)  # [h_local, w]
        out[core * ROWS_PER_CORE : (core + 1) * ROWS_PER_CORE] = sim[:ROWS_PER_CORE]
    return out


_NC_CACHE = {}


def get_nc():
    if "nc" not in _NC_CACHE:
        _NC_CACHE["nc"] = build_bass()
    return _NC_CACHE["nc"]


def kernel(input_image):
    nc = get_nc()
    in_maps = shard_inputs(input_image)
    res = run_bass_kernel_spmd(nc, in_maps, list(range(NCORES)))
    return unshard_output(res.results)


if __name__ == "__main__":
    rng = np.random.default_rng(0)
    x = rng.standard_normal((H, W, 1, C), dtype=np.float32)
    out = kernel(x)
    print(out.shape, out.dtype, out[:2, :4])



# revision 4
# speedup vs baseline: 1.0045x; 1.0045x over previous
"""3x3 neighborhood cosine-similarity sum (minus self) on 8 TRN2 NeuronCores.

Input:  input_image [1024, 1024, 1, 128] float32  (H, W, 1, C)
Output: sim [1024, 1024] float32

Algorithm per pixel: sim = <xn, BoxSum3x3(xn)> - 1, xn = x / max(||x||, eps).

Design (v2):
 - Host casts x to bf16 and pre-gathers an OVERLAPPED chunk layout:
   chunk j (j=0..8) holds w = 126*j + p - 1 for partition p=0..127, i.e.
   consecutive chunks overlap by 2 columns.  Out-of-range w are zero.
   Valid outputs per chunk are p in [1,126].  This makes the horizontal
   3-tap box sum a single constant tridiagonal matmul per chunk with NO
   cross-chunk edge terms, and gives the DMA 2304B contiguous lines.
 - Per input row h (130 per core = 128 + 2 zero-halo rows):
     ss   : 9x fused mult-reduce (DVE)         -> ssr [128, 9] f32
     sqrt : ACT sqrt(ssr + 1e-16)              -> snorm
     recip: DVE reciprocal                     -> sinv
     xn   : 9x tensor_scalar mult (bf16, 4x)   -> xnt
     PE   : band matmuls accumulate T@xn(h) into PSUM S(h-1), S(h), S(h+1)
            (vertical 3-row sum folded into PSUM accumulation; 2 live
            accumulators; one constant tridiagonal bf16 weight forever)
     evac : ACT copy PSUM->SBUF bf16           -> S_sb
     dot  : 9x fused mult-reduce (DVE)         -> simt columns
 - Output rows staged as [blk, p, j*16+rr] f32; host extracts the valid
   (p, j) window and reassembles [H, W].

Sharding: H rows split 128/core across 8 cores, 1 halo row each side.
"""

import os
import sys

import numpy as np
import ml_dtypes

for _p in ("/opt/trn_rl_repo",):
    if _p not in sys.path:
        sys.path.insert(0, _p)

import concourse.bass as bass
import concourse.bacc as bacc
import concourse.mybir as mybir
import concourse.tile as tile
from concourse.bass_utils import run_bass_kernel_spmd

F32 = mybir.dt.float32
BF16 = mybir.dt.bfloat16
ALU = mybir.AluOpType
ACTF = mybir.ActivationFunctionType

H, W, C = 1024, 1024, 128
NCORES = 8
ROWS_PER_CORE = H // NCORES          # 128
NJ = 9                               # overlapped w-chunks, stride 126
WSTRIDE = 126
RBLK = 16                            # output rows per staging block
N_IN = ROWS_PER_CORE + 2             # 130 rows incl zero halos


def build_consts():
    """Tridiagonal band matrix for the horizontal 3-tap box sum."""
    t = np.zeros((128, 128), np.float32)
    for k in range(128):
        for m in (k - 1, k, k + 1):
            if 0 <= m < 128:
                t[k, m] = 1.0
    return t.astype(ml_dtypes.bfloat16)


def build_bass(n_out_rows=ROWS_PER_CORE):
    n_in = n_out_rows + 2
    nblk = (n_out_rows + RBLK - 1) // RBLK
    ss_mode = os.environ.get("SS_MODE", "stt")

    nc = bacc.Bacc(None, target_bir_lowering=False)
    x_dram = nc.declare_dram_parameter("x", [n_in, 128, NJ * C], BF16, isOutput=False)
    band_dram = nc.declare_dram_parameter("band", [128, 128], BF16, isOutput=False)
    out_dram = nc.declare_dram_parameter(
        "out", [nblk, 128, NJ * RBLK], F32, isOutput=True
    )

    with tile.TileContext(nc) as tc:
        with (
            tc.tile_pool(name="consts", bufs=1) as cpool,
            tc.tile_pool(name="xin", bufs=4) as xpool,
            tc.tile_pool(name="sq", bufs=2) as sqpool,
            tc.tile_pool(name="norm", bufs=4) as npool,
            tc.tile_pool(name="xn", bufs=5) as xnpool,
            tc.tile_pool(name="sb", bufs=3) as sbpool,
            tc.tile_pool(name="pd", bufs=2) as pdpool,
            tc.tile_pool(name="sim", bufs=2) as simpool,
            tc.tile_pool(name="psum", bufs=2, space="PSUM") as psumpool,
        ):
            band = cpool.tile([128, 128], BF16, tag="band")
            nc.sync.dma_start(band[:], band_dram[:])
            eps_bias = cpool.tile([128, 1], F32, tag="eps")
            nc.gpsimd.memset(eps_bias[:], 1e-16)

            xn_rows = [None] * n_in
            s_psum = [None] * (n_in + 2)   # S accumulator per output row r
            simt = None

            # matmul sub-splits along the flat free dim (f32 PSUM banks)
            SPLITS = [(0, 4), (4, 8), (8, 9)]   # chunk ranges

            for h in range(n_in):
                # ---- load row h: [128 p, NJ j, 128 c], w = 126*j + p - 1
                xt = xpool.tile([128, NJ, C], BF16, tag="xt")
                nc.sync.dma_start(xt[:], x_dram[h])

                # ---- ss[p, j] = sum_c x^2 (fused per chunk)
                ssr = npool.tile([128, NJ], F32, tag="ssr")
                if ss_mode == "pow":
                    sq = sqpool.tile([128, NJ, C], BF16, tag="sq")
                    for j in range(NJ):
                        nc.vector.tensor_scalar(
                            sq[:, j, :],
                            xt[:, j, :],
                            2.0,
                            None,
                            ALU.pow,
                            accum_out=ssr[:, j : j + 1],
                        )
                else:
                    sq = sqpool.tile([128, NJ, C], BF16, tag="sq")
                    for j in range(NJ):
                        nc.vector.scalar_tensor_tensor(
                            sq[:, j, :],
                            xt[:, j, :],
                            1.0,
                            xt[:, j, :],
                            ALU.mult,
                            ALU.mult,
                            accum_out=ssr[:, j : j + 1],
                        )

                # ---- inv = 1 / sqrt(ss + 1e-16)
                snorm = npool.tile([128, NJ], F32, tag="snorm")
                nc.scalar.activation(snorm[:], ssr[:], ACTF.Sqrt, bias=eps_bias[:])
                sinv = npool.tile([128, NJ], F32, tag="sinv")
                nc.vector.reciprocal(sinv[:], snorm[:])

                # ---- xn = x * inv  (bf16, per-chunk per-partition scalar)
                xnt = xnpool.tile([128, NJ, C], BF16, tag="xnt")
                for j in range(NJ):
                    nc.vector.tensor_scalar(
                        xnt[:, j, :],
                        xt[:, j, :],
                        sinv[:, j : j + 1],
                        None,
                        ALU.mult,
                    )
                xn_rows[h] = xnt

                # ---- PE: accumulate T@xn(h) into S(r) for r in {h-1, h, h+1}
                # S(r) = T @ (xn(r-1) + xn(r) + xn(r+1)), r in [1, n_in-2].
                # Emission order matters: close + evacuate S(h-1) BEFORE
                # allocating S(h+1) so the 2-deep PSUM pool's buffer reuse
                # (WAR on the evac read) is ordered correctly.

                def band_matmuls(S, xn_t, start, stop):
                    for j0, j1 in SPLITS:
                        nc.tensor.matmul(
                            S[:, j0:j1, :],
                            band[:],
                            xn_t[:, j0:j1, :],
                            start=start,
                            stop=stop,
                        )

                r = h - 1
                if 1 <= r <= n_in - 2:
                    # last vertical contribution, then drain this S
                    band_matmuls(s_psum[r], xnt, start=False, stop=True)
                    ro = r - 1  # local output index

                    sb = sbpool.tile([128, NJ, C], BF16, tag="sbt")
                    nc.scalar.activation(sb[:], s_psum[r][:], ACTF.Copy)
                    s_psum[r] = None

                    if ro % RBLK == 0:
                        simt = simpool.tile([128, NJ * RBLK], F32, tag="simt")
                    rr = ro % RBLK
                    pd = pdpool.tile([128, NJ, C], BF16, tag="pd")
                    xb = xn_rows[r]
                    for j in range(NJ):
                        col = j * RBLK + rr
                        nc.vector.scalar_tensor_tensor(
                            pd[:, j, :],
                            xb[:, j, :],
                            1.0,
                            sb[:, j, :],
                            ALU.mult,
                            ALU.mult,
                            accum_out=simt[:, col : col + 1],
                        )
                    xn_rows[r - 1] = None

                    if ro % RBLK == RBLK - 1 or ro == n_out_rows - 1:
                        blk = ro // RBLK
                        simo = simpool.tile([128, NJ * RBLK], F32, tag="simo")
                        nc.vector.tensor_scalar(simo[:], simt[:], -1.0, None, ALU.add)
                        nc.sync.dma_start(out_dram[blk], simo[:])

                if 1 <= h <= n_in - 2:
                    # middle vertical contribution into S(h)
                    band_matmuls(s_psum[h], xnt, start=False, stop=False)

                if 1 <= h + 1 <= n_in - 2:
                    # first vertical contribution opens S(h+1)
                    assert s_psum[h + 1] is None
                    s_psum[h + 1] = psumpool.tile(
                        [128, NJ, C], F32, tag="S", name="S"
                    )
                    band_matmuls(s_psum[h + 1], xnt, start=True, stop=False)

    nc.compile()
    return nc


_GATHER_CACHE = {}


def _overlap_gather_full(x):
    """x [H, W, C] f32 -> [H+2, 128, NJ*C] bf16 with overlapped chunks and
    zero halo rows; w(p, j) = 126*j + p - 1."""
    xb = x.astype(ml_dtypes.bfloat16)
    WPAD = WSTRIDE * (NJ - 1) + 128  # 1136
    xw = np.zeros((H, WPAD, C), ml_dtypes.bfloat16)
    xw[:, 1 : 1 + W] = xb  # shift by 1 so w=-1 maps to index 0 (zeros)
    p = np.arange(128)[:, None]
    j = np.arange(NJ)[None, :]
    widx = WSTRIDE * j + p  # [128, NJ] into padded axis (w+1)
    xov = xw[:, widx, :]  # [H, 128, NJ, C]
    out = np.zeros((H + 2, 128, NJ * C), ml_dtypes.bfloat16)
    out[1 : H + 1] = xov.reshape(H, 128, NJ * C)
    return out


def shard_inputs(input_image):
    x = np.asarray(input_image).reshape(H, W, C).astype(np.float32, copy=False)
    xpad = _overlap_gather_full(x)
    band = build_consts()
    in_maps = []
    for core in range(NCORES):
        lo = core * ROWS_PER_CORE
        shard = np.ascontiguousarray(xpad[lo : lo + N_IN])
        in_maps.append({"x": shard, "band": band})
    return in_maps


def unshard_output(results):
    """results[i]['out'] [nblk, 128, NJ*RBLK] -> [H, W] f32."""
    w = np.arange(W)
    jw = np.minimum(w // WSTRIDE, NJ - 1)
    pw = w - WSTRIDE * jw + 1  # valid partitions 1..126 (chunk 8: 1..16)
    out = np.empty((H, W), np.float32)
    for core in range(NCORES):
        st = np.asarray(results[core]["out"])  # [nblk, 128, NJ*RBLK]
        st = st.reshape(-1, 128, NJ, RBLK)  # [blk, p, j, rr]
        for blk in range(st.shape[0]):
            rows = st[blk][pw, jw, :]  # [W, RBLK]
            h0 = core * ROWS_PER_CORE + blk * RBLK
            out[h0 : h0 + RBLK] = rows.T
    return out


_NC_CACHE = {}


def get_nc():
    if "nc" not in _NC_CACHE:
        _NC_CACHE["nc"] = build_bass()
    return _NC_CACHE["nc"]


def kernel(input_image):
    nc = get_nc()
    in_maps = shard_inputs(input_image)
    res = run_bass_kernel_spmd(nc, in_maps, list(range(NCORES)))
    return unshard_output(res.results)


if __name__ == "__main__":
    rng = np.random.default_rng(0)
    x = rng.standard_normal((H, W, 1, C), dtype=np.float32)
    out = kernel(x)
    print(out.shape, out.dtype, out[:2, :4])


# revision 6
# speedup vs baseline: 1.5287x; 1.5219x over previous
"""3x3 neighborhood cosine-similarity sum (minus self) on 8 TRN2 NeuronCores.

Input:  input_image [1024, 1024, 1, 128] float32  (H, W, 1, C)
Output: sim [1024, 1024] float32

Algorithm per pixel: sim = <xn, BoxSum3x3(xn)> - 1, xn = x / max(||x||, eps).

Design (v3):
 - Host casts x to bf16 and pre-gathers an OVERLAPPED chunk layout:
   chunk j (j=0..8) holds w = 126*j + p - 1 for partition p=0..127
   (chunks overlap by 2 columns; out-of-range w zero; valid outputs are
   p in [1,126] per chunk).  Horizontal 3-tap sum becomes one constant
   tridiagonal matmul per chunk with no cross-chunk terms.
 - Rows processed in blocks of R=4 to amortize DVE instruction overhead:
     sq   : one TT mult (x*x) over [128, R*9*128] bf16
     ss   : in-place halving-tree adds (7 levels) -> [128, R, 9] f32
     sqrt : ACT (batched, +1e-16 bias), recip: DVE (batched)
     xn   : per-(row,chunk) tensor_scalar (bf16, per-partition scalar)
     PE   : band matmuls accumulate T@xn(h) into PSUM S(h-1),S(h),S(h+1)
            (vertical fold in PSUM; close-early ordering; 2 live tiles)
     evac : ACT copy PSUM->SBUF bf16 into the block's S_sb slice
     dot  : one TT mult (xn*S) per block + halving tree -> sim [128,R,9]
 - Output written per block as [rows, 128, 9] f32; host extracts the
   valid (p, j) window and reassembles [H, W].

Sharding: H rows split 128/core across 8 cores, 1 zero halo row each side.
"""

import os
import sys

import numpy as np
import ml_dtypes

for _p in ("/opt/trn_rl_repo",):
    if _p not in sys.path:
        sys.path.insert(0, _p)

import concourse.bass as bass
import concourse.bacc as bacc
import concourse.mybir as mybir
import concourse.tile as tile
from concourse.bass_utils import run_bass_kernel_spmd

F32 = mybir.dt.float32
BF16 = mybir.dt.bfloat16
ALU = mybir.AluOpType
ACTF = mybir.ActivationFunctionType

H, W, C = 1024, 1024, 128
NCORES = 8
ROWS_PER_CORE = H // NCORES          # 128
NJ = 9                               # overlapped w-chunks, stride 126
WSTRIDE = 126
N_IN = ROWS_PER_CORE + 2             # 130 rows incl zero halos
RB = 4                               # rows per batch block


def build_consts():
    t = np.zeros((128, 128), np.float32)
    for k in range(128):
        for m in (k - 1, k, k + 1):
            if 0 <= m < 128:
                t[k, m] = 1.0
    return t.astype(ml_dtypes.bfloat16)


def build_bass(n_out_rows=ROWS_PER_CORE):
    n_in = n_out_rows + 2
    nc = bacc.Bacc(None, target_bir_lowering=False)
    x_dram = nc.declare_dram_parameter("x", [n_in, 128, NJ * C], BF16, isOutput=False)
    band_dram = nc.declare_dram_parameter("band", [128, 128], BF16, isOutput=False)
    out_dram = nc.declare_dram_parameter(
        "out", [n_out_rows, 128, NJ], F32, isOutput=True
    )

    # input row blocks: [h0, h1) ranges of size <= RB
    blocks = [(h0, min(h0 + RB, n_in)) for h0 in range(0, n_in, RB)]

    with tile.TileContext(nc) as tc:
        with (
            tc.tile_pool(name="consts", bufs=1) as cpool,
            tc.tile_pool(name="xin", bufs=3) as xpool,
            tc.tile_pool(name="sq", bufs=2) as sqpool,
            tc.tile_pool(name="norm", bufs=3) as npool,
            tc.tile_pool(name="xn", bufs=3) as xnpool,
            tc.tile_pool(name="sb", bufs=2) as sbpool,
            tc.tile_pool(name="pd", bufs=2) as pdpool,
            tc.tile_pool(name="sim", bufs=2) as simpool,
            tc.tile_pool(name="psum", bufs=2, space="PSUM") as psumpool,
        ):
            band = cpool.tile([128, 128], BF16, tag="band")
            nc.sync.dma_start(band[:], band_dram[:])
            eps_bias = cpool.tile([128, 1], F32, tag="eps")
            nc.gpsimd.memset(eps_bias[:], 1e-16)

            # ring state
            xn_tiles = {}       # block index -> xn tile [128, R, NJ, C]
            s_psum = [None] * (n_in + 2)
            sb_tiles = {}       # block index -> S_sb tile [128, R, NJ, C]

            SPLITS = [(0, 4), (4, 8), (8, 9)]

            def band_matmuls(S, xn_t, start, stop):
                for j0, j1 in SPLITS:
                    nc.tensor.matmul(
                        S[:, j0:j1, :], band[:], xn_t[:, j0:j1, :],
                        start=start, stop=stop,
                    )

            def tree_reduce(big, R, out_f32):
                """big [128, R, NJ, C] bf16, in-place halving tree over C;
                writes [128, R, NJ] sums into out_f32 (an AP)."""
                wdt = C // 2
                nc.vector.tensor_tensor(
                    big[:, :, :, 0:wdt], big[:, :, :, 0:wdt],
                    big[:, :, :, wdt : 2 * wdt], ALU.add,
                )
                while wdt > 2:
                    h = wdt // 2
                    nc.vector.tensor_tensor(
                        big[:, :, :, 0:h], big[:, :, :, 0:h],
                        big[:, :, :, h:wdt], ALU.add,
                    )
                    wdt = h
                nc.vector.tensor_tensor(
                    out_f32, big[:, :, :, 0], big[:, :, :, 1], ALU.add
                )

            def emit_dot_batch(b, h0, h1):
                """dot for rows r in [h0, h1) ∩ [1, n_out]; uses xn block b
                and its S_sb tile; writes sim rows and DMAs out."""
                r0 = max(h0, 1)
                r1 = min(h1, n_out_rows + 1)  # r <= 128
                if r0 >= r1:
                    return
                k0 = r0 - h0
                k1 = r1 - h0
                xb = xn_tiles[b]
                sbt = sb_tiles.pop(b)
                pd = pdpool.tile([128, RB, NJ, C], BF16, tag="pd", name="pd")
                nc.vector.tensor_tensor(
                    pd[:, k0:k1], xb[:, k0:k1], sbt[:, k0:k1], ALU.mult
                )
                simt = simpool.tile([128, RB, NJ], F32, tag="simt", name="simt")
                tree_reduce(pd[:, k0:k1], k1 - k0, simt[:, k0:k1])
                simo = simpool.tile([128, RB, NJ], F32, tag="simo", name="simo")
                nc.vector.tensor_scalar(
                    simo[:, k0:k1], simt[:, k0:k1], -1.0, None, ALU.add
                )
                nc.sync.dma_start(
                    out_dram[r0 - 1 : r1 - 1].rearrange("r p j -> p r j"),
                    simo[:, k0:k1],
                )

            for b, (h0, h1) in enumerate(blocks):
                R = h1 - h0

                xt = xpool.tile([128, RB, NJ, C], BF16, tag="xt", name="xt")
                nc.sync.dma_start(
                    xt[:, 0:R], x_dram[h0:h1].rearrange("r p f -> p r f")
                )

                # ---- squared + tree -> ssr [128, R, NJ]
                sq = sqpool.tile([128, RB, NJ, C], BF16, tag="sq", name="sq")
                nc.vector.tensor_tensor(sq[:, 0:R], xt[:, 0:R], xt[:, 0:R], ALU.mult)
                ssr = npool.tile([128, RB, NJ], F32, tag="ssr", name="ssr")
                tree_reduce(sq[:, 0:R], R, ssr[:, 0:R])

                # ---- inv = 1/sqrt(ss + 1e-16), batched
                snorm = npool.tile([128, RB, NJ], F32, tag="snorm", name="snorm")
                nc.scalar.activation(
                    snorm[:, 0:R], ssr[:, 0:R], ACTF.Sqrt, bias=eps_bias[:]
                )
                sinv = npool.tile([128, RB, NJ], F32, tag="sinv", name="sinv")
                nc.vector.reciprocal(sinv[:, 0:R], snorm[:, 0:R])

                # ---- xn chunks
                xnb = xnpool.tile([128, RB, NJ, C], BF16, tag="xn", name="xn")
                xn_tiles[b] = xnb
                sb_tiles[b] = sbpool.tile(
                    [128, RB, NJ, C], BF16, tag="sbt", name="sbt"
                )

                for k in range(R):
                    h = h0 + k
                    for j in range(NJ):
                        nc.vector.tensor_scalar(
                            xnb[:, k, j, :],
                            xt[:, k, j, :],
                            sinv[:, k, j : j + 1],
                            None,
                            ALU.mult,
                        )
                    xnt = xnb[:, k]

                    # ---- PE vertical-fold band matmuls (close-early order)
                    r = h - 1
                    if 1 <= r <= n_in - 2:
                        band_matmuls(s_psum[r], xnt, start=False, stop=True)
                        # evac into the owning block's S_sb slice
                        rb = r // RB
                        kk = r - rb * RB
                        nc.scalar.activation(
                            sb_tiles[rb][:, kk], s_psum[r][:], ACTF.Copy
                        )
                        s_psum[r] = None
                        # if this closed the last (valid) row of block rb,
                        # emit its dot batch
                        bh0, bh1 = blocks[rb]
                        if r == min(bh1 - 1, n_out_rows):
                            emit_dot_batch(rb, bh0, bh1)

                    if 1 <= h <= n_in - 2:
                        band_matmuls(s_psum[h], xnt, start=False, stop=False)

                    if 1 <= h + 1 <= n_in - 2:
                        assert s_psum[h + 1] is None
                        s_psum[h + 1] = psumpool.tile(
                            [128, NJ, C], F32, tag="S", name="S"
                        )
                        band_matmuls(s_psum[h + 1], xnt, start=True, stop=False)

                # free xn of block b-2 (consumed by its dot batch by now)
                xn_tiles.pop(b - 2, None)

    nc.compile()
    return nc


def _overlap_gather_full(x):
    xb = x.astype(ml_dtypes.bfloat16)
    WPAD = WSTRIDE * (NJ - 1) + 128 + 8
    xw = np.zeros((H, WPAD, C), ml_dtypes.bfloat16)
    xw[:, 1 : 1 + W] = xb
    p = np.arange(128)[:, None]
    j = np.arange(NJ)[None, :]
    widx = WSTRIDE * j + p
    xov = xw[:, widx, :]  # [H, 128, NJ, C]
    out = np.zeros((H + 2, 128, NJ * C), ml_dtypes.bfloat16)
    out[1 : H + 1] = xov.reshape(H, 128, NJ * C)
    return out


def shard_inputs(input_image):
    x = np.asarray(input_image).reshape(H, W, C).astype(np.float32, copy=False)
    xpad = _overlap_gather_full(x)
    band = build_consts()
    in_maps = []
    for core in range(NCORES):
        lo = core * ROWS_PER_CORE
        shard = np.ascontiguousarray(xpad[lo : lo + N_IN])
        in_maps.append({"x": shard, "band": band})
    return in_maps


def unshard_output(results):
    """results[i]['out'] [128, 128, NJ] (rows, p, j) -> [H, W] f32."""
    w = np.arange(W)
    jw = np.minimum(w // WSTRIDE, NJ - 1)
    pw = w - WSTRIDE * jw + 1
    out = np.empty((H, W), np.float32)
    for core in range(NCORES):
        st = np.asarray(results[core]["out"])  # [rows, 128, NJ]
        h0 = core * ROWS_PER_CORE
        out[h0 : h0 + ROWS_PER_CORE] = st[:, pw, jw]
    return out


_NC_CACHE = {}


def get_nc():
    if "nc" not in _NC_CACHE:
        _NC_CACHE["nc"] = build_bass()
    return _NC_CACHE["nc"]


def kernel(input_image):
    nc = get_nc()
    in_maps = shard_inputs(input_image)
    res = run_bass_kernel_spmd(nc, in_maps, list(range(NCORES)))
    return unshard_output(res.results)


if __name__ == "__main__":
    rng = np.random.default_rng(0)
    x = rng.standard_normal((H, W, 1, C), dtype=np.float32)
    out = kernel(x)
    print(out.shape, out.dtype, out[:2, :4])


# revision 8
# speedup vs baseline: 1.8525x; 1.2118x over previous
"""3x3 neighborhood cosine-similarity sum (minus self) on 8 TRN2 NeuronCores.

Input:  input_image [1024, 1024, 1, 128] float32  (H, W, 1, C)
Output: sim [1024, 1024] float32

Algorithm per pixel: sim = <xn, BoxSum3x3(xn)> - 1, xn = x / max(||x||, eps).

Design (v3):
 - Host casts x to bf16 and pre-gathers an OVERLAPPED chunk layout:
   chunk j (j=0..8) holds w = 126*j + p - 1 for partition p=0..127
   (chunks overlap by 2 columns; out-of-range w zero; valid outputs are
   p in [1,126] per chunk).  Horizontal 3-tap sum becomes one constant
   tridiagonal matmul per chunk with no cross-chunk terms.
 - Rows processed in blocks of R=4 to amortize DVE instruction overhead:
     sq   : one TT mult (x*x) over [128, R*9*128] bf16
     ss   : in-place halving-tree adds (7 levels) -> [128, R, 9] f32
     sqrt : ACT (batched, +1e-16 bias), recip: DVE (batched)
     xn   : per-(row,chunk) tensor_scalar (bf16, per-partition scalar)
     PE   : band matmuls accumulate T@xn(h) into PSUM S(h-1),S(h),S(h+1)
            (vertical fold in PSUM; close-early ordering; 2 live tiles)
     evac : ACT copy PSUM->SBUF bf16 into the block's S_sb slice
     dot  : one TT mult (xn*S) per block + halving tree -> sim [128,R,9]
 - Output written per block as [rows, 128, 9] f32; host extracts the
   valid (p, j) window and reassembles [H, W].

Sharding: H rows split 128/core across 8 cores, 1 zero halo row each side.
"""

import os
import sys

import numpy as np
import ml_dtypes

for _p in ("/opt/trn_rl_repo",):
    if _p not in sys.path:
        sys.path.insert(0, _p)

import concourse.bass as bass
import concourse.bacc as bacc
import concourse.mybir as mybir
import concourse.tile as tile
from concourse.bass_utils import run_bass_kernel_spmd

F32 = mybir.dt.float32
BF16 = mybir.dt.bfloat16
ALU = mybir.AluOpType
ACTF = mybir.ActivationFunctionType

H, W, C = 1024, 1024, 128
NCORES = 8
ROWS_PER_CORE = H // NCORES          # 128
NJ = 9                               # overlapped w-chunks, stride 126
WSTRIDE = 126
N_IN = ROWS_PER_CORE + 2             # 130 rows incl zero halos
RB = 6                               # rows per batch block
XN_ACT_CHUNKS = 4                    # xn chunks offloaded to the Scalar engine


def build_consts():
    t = np.zeros((128, 128), np.float32)
    for k in range(128):
        for m in (k - 1, k, k + 1):
            if 0 <= m < 128:
                t[k, m] = 1.0
    return t.astype(ml_dtypes.bfloat16)


def build_bass(n_out_rows=ROWS_PER_CORE):
    n_in = n_out_rows + 2
    nc = bacc.Bacc(None, target_bir_lowering=False)
    x_dram = nc.declare_dram_parameter("x", [n_in, 128, NJ * C], BF16, isOutput=False)
    band_dram = nc.declare_dram_parameter("band", [128, 128], BF16, isOutput=False)
    out_dram = nc.declare_dram_parameter(
        "out", [n_out_rows, 128, NJ], F32, isOutput=True
    )

    # input row blocks: [h0, h1) ranges of size <= RB
    blocks = [(h0, min(h0 + RB, n_in)) for h0 in range(0, n_in, RB)]

    with tile.TileContext(nc) as tc:
        with (
            tc.tile_pool(name="consts", bufs=1) as cpool,
            tc.tile_pool(name="xin", bufs=3) as xpool,
            tc.tile_pool(name="sq", bufs=2) as sqpool,
            tc.tile_pool(name="norm", bufs=3) as npool,
            tc.tile_pool(name="xn", bufs=3) as xnpool,
            tc.tile_pool(name="sb", bufs=2) as sbpool,
            tc.tile_pool(name="pd", bufs=2) as pdpool,
            tc.tile_pool(name="sim", bufs=2) as simpool,
            tc.tile_pool(name="psum", bufs=2, space="PSUM") as psumpool,
        ):
            band = cpool.tile([128, 128], BF16, tag="band")
            nc.sync.dma_start(band[:], band_dram[:])
            eps_bias = cpool.tile([128, 1], F32, tag="eps")
            nc.gpsimd.memset(eps_bias[:], 1e-16)

            # ring state
            xn_tiles = {}       # block index -> xn tile [128, R, NJ, C]
            s_psum = [None] * (n_in + 2)
            sb_tiles = {}       # block index -> S_sb tile [128, R, NJ, C]

            SPLITS = [(0, 4), (4, 8), (8, 9)]

            def band_matmuls(S, xn_t, start, stop):
                for j0, j1 in SPLITS:
                    nc.tensor.matmul(
                        S[:, j0:j1, :], band[:], xn_t[:, j0:j1, :],
                        start=start, stop=stop,
                    )

            def tree_reduce(big, R, out_f32):
                """big [128, R, NJ, C] bf16, in-place halving tree over C;
                writes [128, R, NJ] sums into out_f32 (an AP)."""
                wdt = C // 2
                nc.vector.tensor_tensor(
                    big[:, :, :, 0:wdt], big[:, :, :, 0:wdt],
                    big[:, :, :, wdt : 2 * wdt], ALU.add,
                )
                while wdt > 2:
                    h = wdt // 2
                    nc.vector.tensor_tensor(
                        big[:, :, :, 0:h], big[:, :, :, 0:h],
                        big[:, :, :, h:wdt], ALU.add,
                    )
                    wdt = h
                nc.vector.tensor_tensor(
                    out_f32, big[:, :, :, 0], big[:, :, :, 1], ALU.add
                )

            def emit_dot_batch(b, h0, h1):
                """dot for rows r in [h0, h1) ∩ [1, n_out]; uses xn block b
                and its S_sb tile; writes sim rows and DMAs out."""
                r0 = max(h0, 1)
                r1 = min(h1, n_out_rows + 1)  # r <= 128
                if r0 >= r1:
                    return
                k0 = r0 - h0
                k1 = r1 - h0
                xb = xn_tiles[b]
                sbt = sb_tiles.pop(b)
                pd = pdpool.tile([128, RB, NJ, C], BF16, tag="pd", name="pd")
                nc.vector.tensor_tensor(
                    pd[:, k0:k1], xb[:, k0:k1], sbt[:, k0:k1], ALU.mult
                )
                simt = simpool.tile([128, RB, NJ], F32, tag="simt", name="simt")
                tree_reduce(pd[:, k0:k1], k1 - k0, simt[:, k0:k1])
                simo = simpool.tile([128, RB, NJ], F32, tag="simo", name="simo")
                nc.vector.tensor_scalar(
                    simo[:, k0:k1], simt[:, k0:k1], -1.0, None, ALU.add
                )
                nc.sync.dma_start(
                    out_dram[r0 - 1 : r1 - 1].rearrange("r p j -> p r j"),
                    simo[:, k0:k1],
                )

            for b, (h0, h1) in enumerate(blocks):
                R = h1 - h0

                xt = xpool.tile([128, RB, NJ, C], BF16, tag="xt", name="xt")
                nc.sync.dma_start(
                    xt[:, 0:R], x_dram[h0:h1].rearrange("r p f -> p r f")
                )

                # ---- squared + tree -> ssr [128, R, NJ]
                sq = sqpool.tile([128, RB, NJ, C], BF16, tag="sq", name="sq")
                nc.vector.tensor_tensor(sq[:, 0:R], xt[:, 0:R], xt[:, 0:R], ALU.mult)
                ssr = npool.tile([128, RB, NJ], F32, tag="ssr", name="ssr")
                tree_reduce(sq[:, 0:R], R, ssr[:, 0:R])

                # ---- inv = 1/sqrt(ss + 1e-16), batched
                snorm = npool.tile([128, RB, NJ], F32, tag="snorm", name="snorm")
                nc.scalar.activation(
                    snorm[:, 0:R], ssr[:, 0:R], ACTF.Sqrt, bias=eps_bias[:]
                )
                sinv = npool.tile([128, RB, NJ], F32, tag="sinv", name="sinv")
                nc.vector.reciprocal(sinv[:, 0:R], snorm[:, 0:R])

                # ---- xn chunks
                xnb = xnpool.tile([128, RB, NJ, C], BF16, tag="xn", name="xn")
                xn_tiles[b] = xnb
                sb_tiles[b] = sbpool.tile(
                    [128, RB, NJ, C], BF16, tag="sbt", name="sbt"
                )

                for k in range(R):
                    h = h0 + k
                    for j in range(NJ):
                        if j >= NJ - XN_ACT_CHUNKS:
                            nc.scalar.activation(
                                xnb[:, k, j, :],
                                xt[:, k, j, :],
                                ACTF.Copy,
                                bias=0.0,
                                scale=sinv[:, k, j : j + 1],
                            )
                        else:
                            nc.vector.tensor_scalar(
                                xnb[:, k, j, :],
                                xt[:, k, j, :],
                                sinv[:, k, j : j + 1],
                                None,
                                ALU.mult,
                            )
                    xnt = xnb[:, k]

                    # ---- PE vertical-fold band matmuls (close-early order)
                    r = h - 1
                    if 1 <= r <= n_in - 2:
                        band_matmuls(s_psum[r], xnt, start=False, stop=True)
                        # evac into the owning block's S_sb slice
                        rb = r // RB
                        kk = r - rb * RB
                        nc.scalar.activation(
                            sb_tiles[rb][:, kk], s_psum[r][:], ACTF.Copy
                        )
                        s_psum[r] = None
                        # if this closed the last (valid) row of block rb,
                        # emit its dot batch
                        bh0, bh1 = blocks[rb]
                        if r == min(bh1 - 1, n_out_rows):
                            emit_dot_batch(rb, bh0, bh1)

                    if 1 <= h <= n_in - 2:
                        band_matmuls(s_psum[h], xnt, start=False, stop=False)

                    if 1 <= h + 1 <= n_in - 2:
                        assert s_psum[h + 1] is None
                        s_psum[h + 1] = psumpool.tile(
                            [128, NJ, C], F32, tag="S", name="S"
                        )
                        band_matmuls(s_psum[h + 1], xnt, start=True, stop=False)

                # free xn of block b-2 (consumed by its dot batch by now)
                xn_tiles.pop(b - 2, None)

    nc.compile()
    return nc


def _overlap_gather_full(x):
    xb = x.astype(ml_dtypes.bfloat16)
    WPAD = WSTRIDE * (NJ - 1) + 128 + 8
    xw = np.zeros((H, WPAD, C), ml_dtypes.bfloat16)
    xw[:, 1 : 1 + W] = xb
    p = np.arange(128)[:, None]
    j = np.arange(NJ)[None, :]
    widx = WSTRIDE * j + p
    xov = xw[:, widx, :]  # [H, 128, NJ, C]
    out = np.zeros((H + 2, 128, NJ * C), ml_dtypes.bfloat16)
    out[1 : H + 1] = xov.reshape(H, 128, NJ * C)
    return out


def shard_inputs(input_image):
    x = np.asarray(input_image).reshape(H, W, C).astype(np.float32, copy=False)
    xpad = _overlap_gather_full(x)
    band = build_consts()
    in_maps = []
    for core in range(NCORES):
        lo = core * ROWS_PER_CORE
        shard = np.ascontiguousarray(xpad[lo : lo + N_IN])
        in_maps.append({"x": shard, "band": band})
    return in_maps


def unshard_output(results):
    """results[i]['out'] [128, 128, NJ] (rows, p, j) -> [H, W] f32."""
    w = np.arange(W)
    jw = np.minimum(w // WSTRIDE, NJ - 1)
    pw = w - WSTRIDE * jw + 1
    out = np.empty((H, W), np.float32)
    for core in range(NCORES):
        st = np.asarray(results[core]["out"])  # [rows, 128, NJ]
        h0 = core * ROWS_PER_CORE
        out[h0 : h0 + ROWS_PER_CORE] = st[:, pw, jw]
    return out


_NC_CACHE = {}


def get_nc():
    if "nc" not in _NC_CACHE:
        _NC_CACHE["nc"] = build_bass()
    return _NC_CACHE["nc"]


def kernel(input_image):
    nc = get_nc()
    in_maps = shard_inputs(input_image)
    res = run_bass_kernel_spmd(nc, in_maps, list(range(NCORES)))
    return unshard_output(res.results)


if __name__ == "__main__":
    rng = np.random.default_rng(0)
    x = rng.standard_normal((H, W, 1, C), dtype=np.float32)
    out = kernel(x)
    print(out.shape, out.dtype, out[:2, :4])


# revision 12
# speedup vs baseline: 2.0263x; 1.0938x over previous
"""3x3 neighborhood cosine-similarity sum (minus self) on 8 TRN2 NeuronCores.

Input:  input_image [1024, 1024, 1, 128] float32  (H, W, 1, C)
Output: sim [1024, 1024] float32

Algorithm per pixel: sim = <xn, BoxSum3x3(xn)> - 1, xn = x / max(||x||, eps).

Design (v3):
 - Host casts x to bf16 and pre-gathers an OVERLAPPED chunk layout:
   chunk j (j=0..8) holds w = 126*j + p - 1 for partition p=0..127
   (chunks overlap by 2 columns; out-of-range w zero; valid outputs are
   p in [1,126] per chunk).  Horizontal 3-tap sum becomes one constant
   tridiagonal matmul per chunk with no cross-chunk terms.
 - Rows processed in blocks of R=4 to amortize DVE instruction overhead:
     sq   : one TT mult (x*x) over [128, R*9*128] bf16
     ss   : in-place halving-tree adds (7 levels) -> [128, R, 9] f32
     sqrt : ACT (batched, +1e-16 bias), recip: DVE (batched)
     xn   : per-(row,chunk) tensor_scalar (bf16, per-partition scalar)
     PE   : band matmuls accumulate T@xn(h) into PSUM S(h-1),S(h),S(h+1)
            (vertical fold in PSUM; close-early ordering; 2 live tiles)
     evac : ACT copy PSUM->SBUF bf16 into the block's S_sb slice
     dot  : one TT mult (xn*S) per block + halving tree -> sim [128,R,9]
 - Output written per block as [rows, 128, 9] f32; host extracts the
   valid (p, j) window and reassembles [H, W].

Sharding: H rows split 128/core across 8 cores, 1 zero halo row each side.
"""

import os
import sys

import numpy as np
import ml_dtypes

for _p in ("/opt/trn_rl_repo",):
    if _p not in sys.path:
        sys.path.insert(0, _p)

import concourse.bass as bass
import concourse.bacc as bacc
import concourse.mybir as mybir
import concourse.tile as tile
from concourse.bass_utils import run_bass_kernel_spmd

F32 = mybir.dt.float32
BF16 = mybir.dt.bfloat16
ALU = mybir.AluOpType
ACTF = mybir.ActivationFunctionType


def _register_sq_add_sq():
    """Custom DVE op: out = in0^2 + in1^2 (fuses the square pass with the
    first halving-tree level of the sum-of-squares reduction)."""
    import concourse.dve_ops as dvo
    from concourse.dve_spec import Spec, Src0, Src1, sq

    name = "SQ_ADD_SQ_ANT"
    for op in dvo.OPS:
        if op.name == name:
            return op
    spec = Spec(
        body=sq(Src0) + sq(Src1),
        reference=lambda in0, in1, s0, s1, imm2: (
            np.asarray(in0, np.float32) ** 2 + np.asarray(in1, np.float32) ** 2
        ),
    )
    dvo._SUB_OPCODE_FOR_NAME[name] = max(dvo._SUB_OPCODE_FOR_NAME.values()) + 1
    op = dvo.DveOp(
        name,
        spec,
        subdim=False,
        uops_sha={"v3": "cd4bd6e1c27efd14", "v4": "121e32d8332f5047"},
    )
    dvo.OPS.append(op)
    dvo.CUSTOM_DVE_SPECS[name] = spec
    return op


SQOP = _register_sq_add_sq()

H, W, C = 1024, 1024, 128
NCORES = 8
ROWS_PER_CORE = H // NCORES          # 128
NJ = 9                               # overlapped w-chunks, stride 126
WSTRIDE = 126
N_IN = ROWS_PER_CORE + 2             # 130 rows incl zero halos
RB = 6                               # rows per batch block
XN_ACT_CHUNKS = 4                    # xn chunks offloaded to the Scalar engine


def build_consts():
    t = np.zeros((128, 128), np.float32)
    for k in range(128):
        for m in (k - 1, k, k + 1):
            if 0 <= m < 128:
                t[k, m] = 1.0
    return t.astype(ml_dtypes.bfloat16)


def build_bass(n_out_rows=ROWS_PER_CORE):
    n_in = n_out_rows + 2
    nc = bacc.Bacc(None, target_bir_lowering=False)
    x_dram = nc.declare_dram_parameter("x", [n_in, 128, NJ * C], BF16, isOutput=False)
    band_dram = nc.declare_dram_parameter("band", [128, 128], BF16, isOutput=False)
    out_dram = nc.declare_dram_parameter(
        "out", [n_out_rows, 128, NJ], F32, isOutput=True
    )

    # input row blocks: [h0, h1) ranges of size <= RB
    blocks = [(h0, min(h0 + RB, n_in)) for h0 in range(0, n_in, RB)]

    with tile.TileContext(nc) as tc:
        with (
            tc.tile_pool(name="consts", bufs=1) as cpool,
            tc.tile_pool(name="xin", bufs=3) as xpool,
            tc.tile_pool(name="sq", bufs=2) as sqpool,
            tc.tile_pool(name="norm", bufs=3) as npool,
            tc.tile_pool(name="xn", bufs=3) as xnpool,
            tc.tile_pool(name="sb", bufs=2) as sbpool,
            tc.tile_pool(name="pd", bufs=2) as pdpool,
            tc.tile_pool(name="sim", bufs=2) as simpool,
            tc.tile_pool(name="psum", bufs=2, space="PSUM") as psumpool,
        ):
            band = cpool.tile([128, 128], BF16, tag="band")
            nc.sync.dma_start(band[:], band_dram[:])
            eps_bias = cpool.tile([128, 1], F32, tag="eps")
            nc.gpsimd.memset(eps_bias[:], 1e-16)

            # ring state
            xn_tiles = {}       # block index -> xn tile [128, R, NJ, C]
            s_psum = [None] * (n_in + 2)
            sb_tiles = {}       # block index -> S_sb tile [128, R, NJ, C]

            SPLITS = [(0, 4), (4, 8), (8, 9)]

            def band_matmuls(S, xn_t, start, stop):
                for j0, j1 in SPLITS:
                    nc.tensor.matmul(
                        S[:, j0:j1, :], band[:], xn_t[:, j0:j1, :],
                        start=start, stop=stop,
                    )

            def tree_levels(big3, wdt, out_ap, final_scalar=None):
                """big3 [128, n, wdt] bf16 (3D view), in-place halving tree;
                final level writes `out_ap` [128, n] sums (+final_scalar)."""
                while wdt > 2:
                    h = wdt // 2
                    nc.vector.tensor_tensor(
                        big3[:, :, 0:h], big3[:, :, 0:h], big3[:, :, h:wdt],
                        ALU.add,
                    )
                    wdt = h
                if final_scalar is None:
                    nc.vector.tensor_tensor(
                        out_ap, big3[:, :, 0], big3[:, :, 1], ALU.add
                    )
                else:
                    nc.vector.scalar_tensor_tensor(
                        out_ap, big3[:, :, 0], final_scalar, big3[:, :, 1],
                        ALU.add, ALU.add,
                    )

            def emit_dot_batch(b, h0, h1):
                """dot for rows r in [h0, h1) ∩ [1, n_out]; uses xn block b
                and its S_sb tile; writes sim rows and DMAs out."""
                r0 = max(h0, 1)
                r1 = min(h1, n_out_rows + 1)  # r <= 128
                if r0 >= r1:
                    return
                k0 = r0 - h0
                k1 = r1 - h0
                n0, n1 = k0 * NJ, k1 * NJ
                xbf = xn_tiles[b].rearrange("p r j c -> p (r j) c")
                sbf = sb_tiles.pop(b).rearrange("p r j c -> p (r j) c")
                pd = pdpool.tile([128, RB * NJ, C], BF16, tag="pd", name="pd")
                nc.vector.tensor_tensor(
                    pd[:, n0:n1], xbf[:, n0:n1], sbf[:, n0:n1], ALU.mult
                )
                simo = simpool.tile([128, RB, NJ], F32, tag="simo", name="simo")
                simof = simo.rearrange("p r j -> p (r j)")
                # final level fuses the "- 1" (self-similarity) term
                tree_levels(pd[:, n0:n1], C, simof[:, n0:n1], final_scalar=-1.0)
                nc.sync.dma_start(
                    out_dram[r0 - 1 : r1 - 1].rearrange("r p j -> p r j"),
                    simo[:, k0:k1],
                )

            for b, (h0, h1) in enumerate(blocks):
                R = h1 - h0

                xt = xpool.tile([128, RB, NJ, C], BF16, tag="xt", name="xt")
                nc.sync.dma_start(
                    xt[:, 0:R], x_dram[h0:h1].rearrange("r p f -> p r f")
                )

                # ---- fused square + first tree level -> sq [128, R*NJ, 64]
                n = R * NJ
                xtf = xt.rearrange("p r j c -> p (r j) c")
                sq = sqpool.tile([128, RB * NJ, C // 2], BF16, tag="sq", name="sq")
                nc.vector._custom_dve(
                    SQOP,
                    out=sq[:, 0:n],
                    in0=xtf[:, 0:n, 0 : C // 2],
                    in1=xtf[:, 0:n, C // 2 : C],
                )
                ssr = npool.tile([128, RB, NJ], F32, tag="ssr", name="ssr")
                ssrf = ssr.rearrange("p r j -> p (r j)")
                tree_levels(sq[:, 0:n], C // 2, ssrf[:, 0:n])

                # ---- inv = 1/sqrt(ss + 1e-16), batched
                snorm = npool.tile([128, RB, NJ], F32, tag="snorm", name="snorm")
                nc.scalar.activation(
                    snorm[:, 0:R], ssr[:, 0:R], ACTF.Sqrt, bias=eps_bias[:]
                )
                sinv = npool.tile([128, RB, NJ], F32, tag="sinv", name="sinv")
                nc.vector.reciprocal(sinv[:, 0:R], snorm[:, 0:R])

                # ---- xn chunks
                xnb = xnpool.tile([128, RB, NJ, C], BF16, tag="xn", name="xn")
                xn_tiles[b] = xnb
                sb_tiles[b] = sbpool.tile(
                    [128, RB, NJ, C], BF16, tag="sbt", name="sbt"
                )

                for k in range(R):
                    h = h0 + k
                    for j in range(NJ):
                        if j >= NJ - XN_ACT_CHUNKS:
                            nc.scalar.activation(
                                xnb[:, k, j, :],
                                xt[:, k, j, :],
                                ACTF.Copy,
                                bias=0.0,
                                scale=sinv[:, k, j : j + 1],
                            )
                        else:
                            nc.vector.tensor_scalar(
                                xnb[:, k, j, :],
                                xt[:, k, j, :],
                                sinv[:, k, j : j + 1],
                                None,
                                ALU.mult,
                            )
                    xnt = xnb[:, k]

                    # ---- PE vertical-fold band matmuls (close-early order)
                    r = h - 1
                    if 1 <= r <= n_in - 2:
                        band_matmuls(s_psum[r], xnt, start=False, stop=True)
                        # evac into the owning block's S_sb slice
                        rb = r // RB
                        kk = r - rb * RB
                        nc.scalar.activation(
                            sb_tiles[rb][:, kk], s_psum[r][:], ACTF.Copy
                        )
                        s_psum[r] = None
                        # if this closed the last (valid) row of block rb,
                        # emit its dot batch
                        bh0, bh1 = blocks[rb]
                        if r == min(bh1 - 1, n_out_rows):
                            emit_dot_batch(rb, bh0, bh1)

                    if 1 <= h <= n_in - 2:
                        band_matmuls(s_psum[h], xnt, start=False, stop=False)

                    if 1 <= h + 1 <= n_in - 2:
                        assert s_psum[h + 1] is None
                        s_psum[h + 1] = psumpool.tile(
                            [128, NJ, C], F32, tag="S", name="S"
                        )
                        band_matmuls(s_psum[h + 1], xnt, start=True, stop=False)

                # free xn of block b-2 (consumed by its dot batch by now)
                xn_tiles.pop(b - 2, None)

    nc.compile()
    return nc


def _overlap_gather_full(x):
    xb = x.astype(ml_dtypes.bfloat16)
    WPAD = WSTRIDE * (NJ - 1) + 128 + 8
    xw = np.zeros((H, WPAD, C), ml_dtypes.bfloat16)
    xw[:, 1 : 1 + W] = xb
    p = np.arange(128)[:, None]
    j = np.arange(NJ)[None, :]
    widx = WSTRIDE * j + p
    xov = xw[:, widx, :]  # [H, 128, NJ, C]
    out = np.zeros((H + 2, 128, NJ * C), ml_dtypes.bfloat16)
    out[1 : H + 1] = xov.reshape(H, 128, NJ * C)
    return out


def shard_inputs(input_image):
    x = np.asarray(input_image).reshape(H, W, C).astype(np.float32, copy=False)
    xpad = _overlap_gather_full(x)
    band = build_consts()
    in_maps = []
    for core in range(NCORES):
        lo = core * ROWS_PER_CORE
        shard = np.ascontiguousarray(xpad[lo : lo + N_IN])
        in_maps.append({"x": shard, "band": band})
    return in_maps


def unshard_output(results):
    """results[i]['out'] [128, 128, NJ] (rows, p, j) -> [H, W] f32."""
    w = np.arange(W)
    jw = np.minimum(w // WSTRIDE, NJ - 1)
    pw = w - WSTRIDE * jw + 1
    out = np.empty((H, W), np.float32)
    for core in range(NCORES):
        st = np.asarray(results[core]["out"])  # [rows, 128, NJ]
        h0 = core * ROWS_PER_CORE
        out[h0 : h0 + ROWS_PER_CORE] = st[:, pw, jw]
    return out


_NC_CACHE = {}


def get_nc():
    if "nc" not in _NC_CACHE:
        _NC_CACHE["nc"] = build_bass()
    return _NC_CACHE["nc"]


def kernel(input_image):
    nc = get_nc()
    in_maps = shard_inputs(input_image)
    res = run_bass_kernel_spmd(nc, in_maps, list(range(NCORES)))
    return unshard_output(res.results)


if __name__ == "__main__":
    rng = np.random.default_rng(0)
    x = rng.standard_normal((H, W, 1, C), dtype=np.float32)
    out = kernel(x)
    print(out.shape, out.dtype, out[:2, :4])
